# revision 19
# baseline (speedup 1.0000x reference)
"""Trainium2 Bass kernel for nn_EstimatorNetwork (gnn_message_passing).

Mathematical reformulation: each candidate anchor (f_b, n_b) perturbs a shared
linear recurrence by a rank-1 kill, so

    total(b) = S_base - X[f_b, n_b] * U[f_b, n_b]

where X is the forward chain  x_f = K_f * (b_f + W_{f-1} @ x_{f-1})
and   U the backward chain    u_f = 1 + W_f^T (K_{f+1} * u_{f+1}),
K the selected-anchor keep mask, S_base = sum(X).

Device: 8 cores each own 288 rows of both chains. The chain vectors live in a
permuted ("partition-major") global order chosen so the per-frame AllGather
output loads straight into the matmul-stationary SBUF layout [128, 18] with a
single contiguous DMA -- no transpose. The keep-masks and the bias / "+1" are
folded into the streamed weights on the host (bias rides a 19th contraction
tile against a constant e0 stationary), so each frame is just:
19 matmuls -> PSUM->DRAM DMA -> AllGather -> load+cast. Final phase:
indirect-DMA gather of the 1024 candidate (X, U) pairs + rank-1 combine.
"""
import sys

if "/opt/trn_rl_repo" not in sys.path:
    sys.path.insert(0, "/opt/trn_rl_repo")

import numpy as np

import concourse.bass as bass
import concourse.bacc as bacc
import concourse.mybir as mybir
import concourse.tile as tile
from concourse.bass_utils import run_bass_kernel_spmd

NCORES = 8
NBR = 64            # blocks per row (node = row*64 + col)
N = 2304            # nodes per frame
F = 32              # frames
B = 1024            # candidates
RS = N // NCORES    # 288 rows per core per chain
JT = N // 128       # 18 contraction tiles
KT = JT + 1         # +1 bias tile
BC = B // NCORES    # 128 candidates per core

FP32 = mybir.dt.float32
BF16 = mybir.dt.bfloat16
INT32 = mybir.dt.int32

_PROGRAM = None


def _build_program():
    nc = bacc.Bacc("TRN2", target_bir_lowering=False, debug=False,
                   num_devices=NCORES)

    # ---- per-core external inputs ----
    wf_d = nc.dram_tensor("wf", [F - 1, KT * 128, RS], BF16, kind="ExternalInput")
    wb_d = nc.dram_tensor("wb", [F - 1, KT * 128, RS], BF16, kind="ExternalInput")
    init0_d = nc.dram_tensor("init0", [2, RS], BF16, kind="ExternalInput")
    xidx_d = nc.dram_tensor("xidx", [BC, 1], INT32, kind="ExternalInput")
    uidx_d = nc.dram_tensor("uidx", [BC, 1], INT32, kind="ExternalInput")
    out_d = nc.dram_tensor("out", [BC, 1], FP32, kind="ExternalOutput")

    # ---- internal DRAM: AllGather landing tables (permuted layout) ----
    tabx = nc.dram_tensor("tabx", [F * N], BF16)   # tabx[t*N:] = y(x_t)
    tabv = nc.dram_tensor("tabv", [F * N], BF16)   # tabv[t*N:] = y(u_{31-t})

    groups = [list(range(NCORES))]

    with tile.TileContext(nc) as tc:
        with (
            tc.tile_pool(name="const", bufs=1) as cpool,
            tc.tile_pool(name="wpool", bufs=2) as wpool,
            tc.tile_pool(name="sb", bufs=2) as sb,
            tc.tile_pool(name="ps", bufs=2, space="PSUM") as ps,
            tc.tile_pool(name="ps1", bufs=1, space="PSUM") as ps1,
            tc.tile_pool(name="agdram", bufs=2, space="DRAM") as agdram,
        ):
            # constants: e0 = one-hot(partition 0) stationary for the bias tile
            e0 = cpool.tile([128, 1], BF16, tag="e0")
            nc.gpsimd.memset(e0[:], 0.0)
            nc.gpsimd.memset(e0[0:1, :], 1.0)
            acc = cpool.tile([128, JT], FP32, tag="acc")
            nc.gpsimd.memset(acc[:], 0.0)

            xS = None   # bf16 stationary [128, JT] for fwd step t+1
            vS = None

            def matvec(w_tile, stat, tag):
                # pack 2 independent accumulation chains into 2 PE column
                # strips; paired matmuls run concurrently (fill/drain overlap)
                psr = ps.tile([33, RS], FP32, tag=tag)
                nslots = [0, 0]
                for j in range(KT):
                    nslots[j % 2] += 1
                seen = [0, 0]
                for j in range(KT):
                    g = j % 2
                    seen[g] += 1
                    lhs = e0[:] if j == JT else stat[:, j:j + 1]
                    nc.tensor.matmul(
                        psr[32 * g:32 * g + 1, :], lhs, w_tile[:, j, :],
                        start=(seen[g] == 1), stop=(seen[g] == nslots[g]),
                        tile_position=(0, 32 * g),
                    )
                return psr

            def prep(tab, t, dma_eng, tag, want_acc):
                nat = sb.tile([128, JT], BF16, tag=tag + "_nat")
                dma_eng.dma_start(
                    nat[:],
                    tab[t * N:(t + 1) * N].rearrange("(p j) -> p j", p=128),
                )
                if want_acc:
                    nc.vector.tensor_add(acc[:], acc[:], nat[:])
                if t == F - 1:
                    return None
                return nat

            for t in range(F):
                # ---------- forward ----------
                aginx = agdram.tile([1, RS], BF16, tag="aginx")
                if t == 0:
                    nc.scalar.dma_start(aginx[:], init0_d[0].unsqueeze(0))
                else:
                    wf_t = wpool.tile([128, KT, RS], BF16, tag="wf")
                    nc.sync.dma_start(
                        wf_t[:], wf_d[t - 1].rearrange("(j p) n -> p j n", p=128)
                    )
                    psx = matvec(wf_t, xS, "psx")
                    xsl = sb.tile([1, RS], BF16, tag="xsl")
                    xt2 = sb.tile([1, RS], FP32, tag="xt2")
                    nc.vector.tensor_copy(xt2[:], psx[0:1, :])
                    nc.vector.tensor_add(xsl[:], xt2[:], psx[32:33, :])
                    nc.scalar.dma_start(aginx[:], xsl[:])
                nc.gpsimd.collective_compute(
                    "AllGather", mybir.AluOpType.bypass, replica_groups=groups,
                    ins=[aginx[:]], outs=[tabx[t * N:(t + 1) * N]],
                )

                # vS for this round's bwd matvec (gathered last round); sits
                # between the fwd and bwd MM streams on the PE queue
                if t > 0:
                    vS = prep(tabv, t - 1, nc.scalar, "v", want_acc=False)

                # ---------- backward ----------
                aginv = agdram.tile([1, RS], BF16, tag="aginv")
                if t == 0:
                    nc.scalar.dma_start(aginv[:], init0_d[1].unsqueeze(0))
                else:
                    wb_t = wpool.tile([128, KT, RS], BF16, tag="wb")
                    nc.sync.dma_start(
                        wb_t[:], wb_d[t - 1].rearrange("(j p) n -> p j n", p=128)
                    )
                    psv = matvec(wb_t, vS, "psv")
                    vsl = sb.tile([1, RS], BF16, tag="vsl")
                    vt2 = sb.tile([1, RS], FP32, tag="vt2")
                    nc.vector.tensor_copy(vt2[:], psv[0:1, :])
                    nc.vector.tensor_add(vsl[:], vt2[:], psv[32:33, :])
                    nc.scalar.dma_start(aginv[:], vsl[:])
                nc.gpsimd.collective_compute(
                    "AllGather", mybir.AluOpType.bypass, replica_groups=groups,
                    ins=[aginv[:]], outs=[tabv[t * N:(t + 1) * N]],
                )

                # xS for next round's fwd matvec
                xS = prep(tabx, t, nc.scalar, "x", want_acc=True)

            # ---------- finale: S_base broadcast + candidate gather ----------
            red = sb.tile([128, 1], FP32, tag="red")
            nc.vector.tensor_reduce(red[:], acc[:], mybir.AxisListType.X,
                                    mybir.AluOpType.add)
            ones = cpool.tile([128, 128], FP32, tag="ones")
            nc.gpsimd.memset(ones[:], 1.0)
            ps_sb = ps1.tile([128, 1], FP32, tag="ps_sb")
            nc.tensor.matmul(ps_sb[:], ones[:], red[:], start=True, stop=True)

            idx_x = sb.tile([BC, 1], INT32, tag="idx_x")
            idx_u = sb.tile([BC, 1], INT32, tag="idx_u")
            nc.sync.dma_start(idx_x[:], xidx_d[:])
            nc.sync.dma_start(idx_u[:], uidx_d[:])
            gx = sb.tile([BC, 1], BF16, tag="gx")
            gu = sb.tile([BC, 1], BF16, tag="gu")
            nc.gpsimd.indirect_dma_start(
                out=gx[:], out_offset=None,
                in_=tabx[:].rearrange("(a b) -> a b", b=1),
                in_offset=bass.IndirectOffsetOnAxis(ap=idx_x[:, :1], axis=0),
            )
            nc.gpsimd.indirect_dma_start(
                out=gu[:], out_offset=None,
                in_=tabv[:].rearrange("(a b) -> a b", b=1),
                in_offset=bass.IndirectOffsetOnAxis(ap=idx_u[:, :1], axis=0),
            )
            prod = sb.tile([BC, 1], FP32, tag="prod")
            nc.vector.tensor_mul(prod[:], gx[:], gu[:])
            outv = sb.tile([BC, 1], FP32, tag="outv")
            nc.vector.tensor_sub(outv[:], ps_sb[:], prod[:])
            nc.sync.dma_start(out_d[:], outv[:])

    nc.compile()
    return nc


def _get_program():
    global _PROGRAM
    if _PROGRAM is None:
        _PROGRAM = _build_program()
    return _PROGRAM


def _host_prep(weights, biases, selected_anchor_points, candidate_anchor_points):
    import ml_dtypes
    BF = ml_dtypes.bfloat16

    W = np.ascontiguousarray(weights, dtype=np.float32)
    Bi = np.ascontiguousarray(biases, dtype=np.float32)
    sel = np.asarray(selected_anchor_points)
    cand = np.asarray(candidate_anchor_points)

    K = np.ones((F, N), dtype=np.float32)
    K[sel[:, 0], sel[:, 1] * NBR + sel[:, 2]] = 0.0

    # permuted global order: position q = l*18 + j  <->  x-row i = 128*j + l
    i_of_q = 128 * (np.arange(N) % JT) + np.arange(N) // JT
    perm_pos = np.empty(N, dtype=np.int64)   # x-row -> table position
    perm_pos[i_of_q] = np.arange(N)

    cf = cand[:, 0].astype(np.int64)
    cn = (cand[:, 1] * NBR + cand[:, 2]).astype(np.int64)
    xidx = (cf * N + perm_pos[cn]).astype(np.int32)
    uidx = ((F - 1 - cf) * N + perm_pos[cn]).astype(np.int32)

    # fwd: x_f = K_f*(b_f + W[f-1] x_{f-1}) -> rows of W[f-1] masked by K_f
    Wfm = W * K[1:, :, None]
    # bwd: u prev <- W[31-t]^T (K[32-t] * u): contraction rows masked
    Wrev = W[::-1]
    Kpre = K[F - 1:0:-1]
    Wbm = Wrev * Kpre[:, :, None]
    bK = Bi * K

    in_maps = []
    for c in range(NCORES):
        i_out = i_of_q[RS * c: RS * (c + 1)]   # the 288 x-rows this core owns
        # fwd slab [31, KT*128, RS]: contraction row k of tile j = x-row 128j+k
        wf_c = np.zeros((F - 1, KT * 128, RS), dtype=BF)
        wf_c[:, :N, :] = Wfm[:, i_out, :].transpose(0, 2, 1).astype(BF)
        wf_c[:, N, :] = bK[1:, i_out].astype(BF)          # bias row (e0 tile)
        # bwd slab: contraction row = u-input row; outputs = same i_out cols
        wb_c = np.zeros((F - 1, KT * 128, RS), dtype=BF)
        wb_c[:, :N, :] = Wbm[:, :, i_out].astype(BF)
        wb_c[:, N, :] = 1.0                               # the "+1"
        init0 = np.stack([bK[0, i_out],
                          np.ones(RS, dtype=np.float32)]).astype(BF)
        in_maps.append({
            "wf": wf_c,
            "wb": wb_c,
            "init0": init0,
            "xidx": xidx[BC * c: BC * (c + 1)].reshape(BC, 1),
            "uidx": uidx[BC * c: BC * (c + 1)].reshape(BC, 1),
        })
    return in_maps


def kernel(weights, biases, selected_anchor_points, candidate_anchor_points):
    nc = _get_program()
    in_maps = _host_prep(weights, biases, selected_anchor_points,
                         candidate_anchor_points)
    last_err = None
    for _attempt in range(2):
        try:
            res = run_bass_kernel_spmd(nc, in_maps,
                                       core_ids=list(range(NCORES)))
            break
        except Exception as e:  # transient device flake: retry once
            last_err = e
    else:
        raise last_err
    out = np.concatenate(
        [res.results[c]["out"].reshape(BC) for c in range(NCORES)]
    ).astype(np.float32)
    return out


# revision 20
# speedup vs baseline: 1.0181x; 1.0181x over previous
"""Trainium2 Bass kernel for nn_EstimatorNetwork (gnn_message_passing).

Mathematical reformulation: each candidate anchor (f_b, n_b) perturbs a shared
linear recurrence by a rank-1 kill, so

    total(b) = S_base - X[f_b, n_b] * U[f_b, n_b]

where X is the forward chain  x_f = K_f * (b_f + W_{f-1} @ x_{f-1})
and   U the backward chain    u_f = 1 + W_f^T (K_{f+1} * u_{f+1}),
K the selected-anchor keep mask, S_base = sum(X).

Device: 8 cores each own 288 rows of both chains. The chain vectors live in a
permuted ("partition-major") global order chosen so the per-frame AllGather
output loads straight into the matmul-stationary SBUF layout [128, 18] with a
single contiguous DMA -- no transpose. The keep-masks and the bias / "+1" are
folded into the streamed weights on the host (bias rides a 19th contraction
tile against a constant e0 stationary), so each frame is just:
19 matmuls -> PSUM->DRAM DMA -> AllGather -> load+cast. Final phase:
indirect-DMA gather of the 1024 candidate (X, U) pairs + rank-1 combine.
"""
import sys

if "/opt/trn_rl_repo" not in sys.path:
    sys.path.insert(0, "/opt/trn_rl_repo")

import numpy as np

import concourse.bass as bass
import concourse.bacc as bacc
import concourse.mybir as mybir
import concourse.tile as tile
from concourse.bass_utils import run_bass_kernel_spmd

NCORES = 8
NBR = 64            # blocks per row (node = row*64 + col)
N = 2304            # nodes per frame
F = 32              # frames
B = 1024            # candidates
RS = N // NCORES    # 288 rows per core per chain
JT = N // 128       # 18 contraction tiles
KT = JT + 1         # +1 bias tile
BC = B // NCORES    # 128 candidates per core

FP32 = mybir.dt.float32
BF16 = mybir.dt.bfloat16
INT32 = mybir.dt.int32

_PROGRAM = None


def _build_program():
    nc = bacc.Bacc("TRN2", target_bir_lowering=False, debug=False,
                   num_devices=NCORES)

    # ---- per-core external inputs ----
    wf_d = nc.dram_tensor("wf", [F - 1, KT * 128, RS], BF16, kind="ExternalInput")
    wb_d = nc.dram_tensor("wb", [F - 1, KT * 128, RS], BF16, kind="ExternalInput")
    init0_d = nc.dram_tensor("init0", [2, RS], BF16, kind="ExternalInput")
    xidx_d = nc.dram_tensor("xidx", [BC, 1], INT32, kind="ExternalInput")
    uidx_d = nc.dram_tensor("uidx", [BC, 1], INT32, kind="ExternalInput")
    out_d = nc.dram_tensor("out", [BC, 1], FP32, kind="ExternalOutput")

    # ---- internal DRAM: AllGather landing tables (permuted layout) ----
    tabx = nc.dram_tensor("tabx", [F * N], BF16)   # tabx[t*N:] = y(x_t)
    tabv = nc.dram_tensor("tabv", [F * N], BF16)   # tabv[t*N:] = y(u_{31-t})

    groups = [list(range(NCORES))]

    with tile.TileContext(nc) as tc:
        with (
            tc.tile_pool(name="const", bufs=1) as cpool,
            tc.tile_pool(name="wpool", bufs=2) as wpool,
            tc.tile_pool(name="sb", bufs=2) as sb,
            tc.tile_pool(name="ps", bufs=2, space="PSUM") as ps,
            tc.tile_pool(name="ps1", bufs=1, space="PSUM") as ps1,
            tc.tile_pool(name="agdram", bufs=2, space="DRAM") as agdram,
        ):
            # constants: e0 = one-hot(partition 0) stationary for the bias tile
            e0 = cpool.tile([128, 1], BF16, tag="e0")
            nc.gpsimd.memset(e0[:], 0.0)
            nc.gpsimd.memset(e0[0:1, :], 1.0)
            acc = cpool.tile([128, JT], FP32, tag="acc")
            nc.gpsimd.memset(acc[:], 0.0)

            xS = None   # bf16 stationary [128, JT] for fwd step t+1
            vS = None

            def matvec(w_tile, stat, tag):
                # pack 2 independent accumulation chains into 2 PE column
                # strips; paired matmuls run concurrently (fill/drain overlap)
                psr = ps.tile([33, RS], FP32, tag=tag)
                nslots = [0, 0]
                for j in range(KT):
                    nslots[j % 2] += 1
                seen = [0, 0]
                for j in range(KT):
                    g = j % 2
                    seen[g] += 1
                    lhs = e0[:] if j == JT else stat[:, j:j + 1]
                    nc.tensor.matmul(
                        psr[32 * g:32 * g + 1, :], lhs, w_tile[:, j, :],
                        start=(seen[g] == 1), stop=(seen[g] == nslots[g]),
                        tile_position=(0, 32 * g),
                    )
                return psr

            def prep(tab, t, dma_eng, tag, want_acc):
                nat = sb.tile([128, JT], BF16, tag=tag + "_nat")
                dma_eng.dma_start(
                    nat[:],
                    tab[t * N:(t + 1) * N].rearrange("(p j) -> p j", p=128),
                )
                if want_acc:
                    nc.vector.tensor_add(acc[:], acc[:], nat[:])
                if t == F - 1:
                    return None
                return nat

            for t in range(F):
                # ---------- forward ----------
                aginx = agdram.tile([1, RS], BF16, tag="aginx")
                if t == 0:
                    nc.scalar.dma_start(aginx[:], init0_d[0].unsqueeze(0))
                else:
                    wf_t = wpool.tile([128, KT, RS], BF16, tag="wf")
                    nc.sync.dma_start(
                        wf_t[:], wf_d[t - 1].rearrange("(j p) n -> p j n", p=128)
                    )
                    psx = matvec(wf_t, xS, "psx")
                    xsl = sb.tile([1, RS], BF16, tag="xsl")
                    xt2 = sb.tile([1, RS], FP32, tag="xt2")
                    nc.vector.tensor_copy(xt2[:], psx[0:1, :])
                    nc.vector.tensor_add(xsl[:], xt2[:], psx[32:33, :])
                    nc.scalar.dma_start(aginx[:], xsl[:])
                nc.gpsimd.collective_compute(
                    "AllGather", mybir.AluOpType.bypass, replica_groups=groups,
                    ins=[aginx[:]], outs=[tabx[t * N:(t + 1) * N]],
                )

                # vS for this round's bwd matvec (gathered last round); sits
                # between the fwd and bwd MM streams on the PE queue
                if t > 0:
                    vS = prep(tabv, t - 1, nc.scalar, "v", want_acc=False)

                # ---------- backward ----------
                aginv = agdram.tile([1, RS], BF16, tag="aginv")
                if t == 0:
                    nc.scalar.dma_start(aginv[:], init0_d[1].unsqueeze(0))
                else:
                    wb_t = wpool.tile([128, KT, RS], BF16, tag="wb")
                    nc.sync.dma_start(
                        wb_t[:], wb_d[t - 1].rearrange("(j p) n -> p j n", p=128)
                    )
                    psv = matvec(wb_t, vS, "psv")
                    vsl = sb.tile([1, RS], BF16, tag="vsl")
                    vt2 = sb.tile([1, RS], FP32, tag="vt2")
                    nc.vector.tensor_copy(vt2[:], psv[0:1, :])
                    nc.vector.tensor_add(vsl[:], vt2[:], psv[32:33, :])
                    nc.scalar.dma_start(aginv[:], vsl[:])
                nc.gpsimd.collective_compute(
                    "AllGather", mybir.AluOpType.bypass, replica_groups=groups,
                    ins=[aginv[:]], outs=[tabv[t * N:(t + 1) * N]],
                )

                # xS for next round's fwd matvec
                xS = prep(tabx, t, nc.scalar, "x", want_acc=True)

            # ---------- finale: S_base broadcast + candidate gather ----------
            red = sb.tile([128, 1], FP32, tag="red")
            nc.vector.tensor_reduce(red[:], acc[:], mybir.AxisListType.X,
                                    mybir.AluOpType.add)
            ones = cpool.tile([128, 128], FP32, tag="ones")
            nc.gpsimd.memset(ones[:], 1.0)
            ps_sb = ps1.tile([128, 1], FP32, tag="ps_sb")
            nc.tensor.matmul(ps_sb[:], ones[:], red[:], start=True, stop=True)

            idx_x = sb.tile([BC, 1], INT32, tag="idx_x")
            idx_u = sb.tile([BC, 1], INT32, tag="idx_u")
            nc.sync.dma_start(idx_x[:], xidx_d[:])
            nc.sync.dma_start(idx_u[:], uidx_d[:])
            gx = sb.tile([BC, 1], BF16, tag="gx")
            gu = sb.tile([BC, 1], BF16, tag="gu")
            nc.gpsimd.indirect_dma_start(
                out=gx[:], out_offset=None,
                in_=tabx[:].rearrange("(a b) -> a b", b=1),
                in_offset=bass.IndirectOffsetOnAxis(ap=idx_x[:, :1], axis=0),
            )
            nc.gpsimd.indirect_dma_start(
                out=gu[:], out_offset=None,
                in_=tabv[:].rearrange("(a b) -> a b", b=1),
                in_offset=bass.IndirectOffsetOnAxis(ap=idx_u[:, :1], axis=0),
            )
            prod = sb.tile([BC, 1], FP32, tag="prod")
            nc.vector.tensor_mul(prod[:], gx[:], gu[:])
            outv = sb.tile([BC, 1], FP32, tag="outv")
            nc.vector.tensor_sub(outv[:], ps_sb[:], prod[:])
            nc.sync.dma_start(out_d[:], outv[:])

    nc.compile()
    return nc


def _get_program():
    global _PROGRAM
    if _PROGRAM is None:
        _PROGRAM = _build_program()
    return _PROGRAM


def _host_prep(weights, biases, selected_anchor_points, candidate_anchor_points):
    import ml_dtypes
    BF = ml_dtypes.bfloat16

    W = np.ascontiguousarray(weights, dtype=np.float32)
    Bi = np.ascontiguousarray(biases, dtype=np.float32)
    sel = np.asarray(selected_anchor_points)
    cand = np.asarray(candidate_anchor_points)

    K = np.ones((F, N), dtype=np.float32)
    K[sel[:, 0], sel[:, 1] * NBR + sel[:, 2]] = 0.0

    # permuted global order: position q = l*18 + j  <->  x-row i = 128*j + l
    i_of_q = 128 * (np.arange(N) % JT) + np.arange(N) // JT
    perm_pos = np.empty(N, dtype=np.int64)   # x-row -> table position
    perm_pos[i_of_q] = np.arange(N)

    cf = cand[:, 0].astype(np.int64)
    cn = (cand[:, 1] * NBR + cand[:, 2]).astype(np.int64)
    xidx = (cf * N + perm_pos[cn]).astype(np.int32)
    uidx = ((F - 1 - cf) * N + perm_pos[cn]).astype(np.int32)

    bK = Bi * K
    in_maps = [{} for _ in range(NCORES)]
    i_outs = [i_of_q[RS * c: RS * (c + 1)] for c in range(NCORES)]

    # fwd: x_f = K_f*(b_f + W[f-1] x_{f-1}) -> rows of W[f-1] masked by K_f
    Wfm = W * K[1:, :, None]
    for c in range(NCORES):
        i_out = i_outs[c]
        # fwd slab [31, KT*128, RS]: contraction row k of tile j = x-row 128j+k
        wf_c = np.zeros((F - 1, KT * 128, RS), dtype=BF)
        wf_c[:, :N, :] = Wfm[:, i_out, :].transpose(0, 2, 1).astype(BF)
        wf_c[:, N, :] = bK[1:, i_out].astype(BF)          # bias row (e0 tile)
        in_maps[c]["wf"] = wf_c
    del Wfm

    # bwd: u prev <- W[31-t]^T (K[32-t] * u): contraction rows masked
    Wbm = W[::-1] * K[F - 1:0:-1][:, :, None]
    for c in range(NCORES):
        i_out = i_outs[c]
        # bwd slab: contraction row = u-input row; outputs = same i_out cols
        wb_c = np.zeros((F - 1, KT * 128, RS), dtype=BF)
        wb_c[:, :N, :] = Wbm[:, :, i_out].astype(BF)
        wb_c[:, N, :] = 1.0                               # the "+1"
        in_maps[c]["wb"] = wb_c
    del Wbm

    for c in range(NCORES):
        i_out = i_outs[c]
        in_maps[c]["init0"] = np.stack(
            [bK[0, i_out], np.ones(RS, dtype=np.float32)]).astype(BF)
        in_maps[c]["xidx"] = xidx[BC * c: BC * (c + 1)].reshape(BC, 1)
        in_maps[c]["uidx"] = uidx[BC * c: BC * (c + 1)].reshape(BC, 1)
    return in_maps


def kernel(weights, biases, selected_anchor_points, candidate_anchor_points):
    nc = _get_program()
    in_maps = _host_prep(weights, biases, selected_anchor_points,
                         candidate_anchor_points)
    last_err = None
    for _attempt in range(2):
        try:
            res = run_bass_kernel_spmd(nc, in_maps,
                                       core_ids=list(range(NCORES)))
            break
        except Exception as e:  # transient device flake: retry once
            last_err = e
    else:
        raise last_err
    out = np.concatenate(
        [res.results[c]["out"].reshape(BC) for c in range(NCORES)]
    ).astype(np.float32)
    return out
